# revision 9
# baseline (speedup 1.0000x reference)
"""Trainium2 Bass kernel for nn_CalculateSLayer (GNN message passing).

Reference computation:
    edge_emb = emb_table[matrix]                        # [n, n, 2, 10]
    hW = h @ W[:60]                                     # [n, 70]
    eW = einsum('ijkd,df->ijkf', edge_emb, W[60:])      # [n, n, 2, 70]
    t  = tanh(hW[:,None,None,:] + eW + b)               # [n, n, 2, 70]
    s  = t * mask
    s_in  = s.sum(axis=(1, 2))                          # [n, 70]
    s_out = s.sum(axis=(0, 2))                          # [n, 70]

Key algebraic restructure: t[i,j,k,:] depends only on (i, matrix[i,j,k]),
taking one of 50 values per row i:  T[i,c,:] = tanh(hW[i]+E[c]+b) with
E = emb_table @ W[60:].  With per-(i,j) masked type counts
N[i,j,c] = sum_k mask*[matrix==c]:
    s_out[j] = sum_{i,c} N[i,j,c] * T[i,c,:]   (big PE matmul)
    s_in[i]  = sum_c hist[i,c] * T[i,c,:],  hist = N summed over j.
On device we build, for each type c, the one-hot plane (z==c) (bf16, DVE
tensor_scalar at 4x rate, with accum_out giving hist for free) and
accumulate T_c^T @ plane into PSUM over all 50 types.

Sharding: rows i are split 128-per-core across 8 cores.  s_in is fully
local per core; s_out partials are summed across cores at gather time.
"""
import os
import sys
import numpy as np

sys.path.insert(0, "/opt/trn_rl_repo")

N = 1024
H2 = 60
DEP = 10
F = 70          # DOUT
NT = 50         # edge types
NCORES = 8
P = 128         # rows per core
JK = 2 * N      # free elements per row (j, k) with k innermost
DEAD = 51.0     # masked-out elements get z = 51, matching no plane

_CACHE = {}


def _build_nc():
    from concourse import bacc, mybir
    from concourse import tile

    f32 = mybir.dt.float32
    bf16 = mybir.dt.bfloat16
    i32 = mybir.dt.int32
    Alu = mybir.AluOpType
    ActF = mybir.ActivationFunctionType

    nc = bacc.Bacc("TRN2", target_bir_lowering=False, debug=False,
                   num_devices=NCORES)

    mat_d = nc.dram_tensor("mat", [P, JK], i32, kind="ExternalInput")
    msk_d = nc.dram_tensor("msk", [P, JK], i32, kind="ExternalInput")
    hx62_d = nc.dram_tensor("hx62", [H2 + 2, P], f32, kind="ExternalInput")
    wstack_d = nc.dram_tensor("wstack", [H2 + 2, NT * F], f32,
                              kind="ExternalInput")

    sin_d = nc.dram_tensor("s_in_part", [P, F], f32, kind="ExternalOutput")
    soutT_d = nc.dram_tensor("s_outT_part", [F, N], f32, kind="ExternalOutput")

    with tile.TileContext(nc) as tc:
        with (
            tc.tile_pool(name="const", bufs=1) as cpool,
            tc.tile_pool(name="work", bufs=2) as wpool,
            tc.tile_pool(name="planes", bufs=4) as ppool,
            tc.tile_pool(name="psmall", bufs=1, space="PSUM") as ps_small,
            tc.tile_pool(name="pbig", bufs=1, space="PSUM") as ps_big,
        ):
            # ---- small weights in ----
            hx62 = cpool.tile([H2 + 2, P], f32, tag="hx62")
            wstack = cpool.tile([H2 + 2, NT * F], f32, tag="wstack")
            nc.sync.dma_start(out=hx62[:], in_=hx62_d[:])
            nc.sync.dma_start(out=wstack[:], in_=wstack_d[:])

            # ---- big inputs in ----
            mat_i = wpool.tile([P, JK], i32, tag="mat_i")
            msk_i = wpool.tile([P, JK], i32, tag="msk_i")
            nc.sync.dma_start(out=mat_i[:], in_=mat_d[:])
            nc.sync.dma_start(out=msk_i[:], in_=msk_d[:])

            # ---- T[i, c, f] = tanh(hW[i,f] + b[f] + E[c,f]), c-outer ----
            # one matmul per type: [1|h|1]^T contracted with [E_c ; W[:60] ; b]
            # 7 types per PSUM bank (7*70=490 <= 512 f32/bank)
            T_sb = cpool.tile([P, NT * F], bf16, tag="T")
            idx = 0
            while idx < NT:
                cnt = min(7, NT - idx)
                t_ps = ps_big.tile([P, 512], f32, tag="big")
                for cl in range(cnt):
                    c = idx + cl
                    nc.tensor.matmul(
                        out=t_ps[:, cl * F:(cl + 1) * F],
                        lhsT=hx62[:], rhs=wstack[:, c * F:(c + 1) * F],
                        start=True, stop=True)
                nc.scalar.activation(
                    out=T_sb[:, idx * F:(idx + cnt) * F],
                    in_=t_ps[:, :cnt * F], func=ActF.Tanh)
                idx += cnt

            # ---- z = mask ? matrix : 51  (bf16) ----
            mat_f = wpool.tile([P, JK], f32, tag="mat_f")
            msk_f = wpool.tile([P, JK], f32, tag="msk_f")
            nc.vector.tensor_copy(out=mat_f[:], in_=mat_i[:])
            nc.vector.tensor_copy(out=msk_f[:], in_=msk_i[:])
            z1 = wpool.tile([P, JK], f32, tag="z1")
            nc.vector.scalar_tensor_tensor(
                out=z1[:], in0=mat_f[:], scalar=DEAD, in1=msk_f[:],
                op0=Alu.subtract, op1=Alu.mult)
            zb = wpool.tile([P, JK], bf16, tag="zb")
            nc.vector.tensor_scalar(
                out=zb[:], in0=z1[:], scalar1=DEAD, scalar2=None,
                op0=Alu.add)

            # ---- main type loop ----
            hist = cpool.tile([P, NT], f32, tag="hist")
            so_ps = ps_big.tile([F, JK], f32, tag="big")
            NQ = 4  # moving-operand slices of 512
            for c in range(NT):
                mc = ppool.tile([P, JK], bf16, tag="mc")
                nc.vector.tensor_scalar(
                    out=mc[:], in0=zb[:], scalar1=float(c), scalar2=None,
                    op0=Alu.is_equal, op1=Alu.add,
                    accum_out=hist[:, c:c + 1])
                lhsT = T_sb[:, c * F:(c + 1) * F]
                for q in range(NQ):
                    nc.tensor.matmul(
                        out=so_ps[:, q * 512:(q + 1) * 512],
                        lhsT=lhsT, rhs=mc[:, q * 512:(q + 1) * 512],
                        start=(c == 0), stop=(c == NT - 1))

            # ---- fold k: s_outT[f, j] = so_ps[f, 2j] + so_ps[f, 2j+1] ----
            so_cp = wpool.tile([F, JK], f32, tag="so_cp")
            nc.vector.tensor_copy(out=so_cp[:], in_=so_ps[:])
            so_v = so_cp[:].rearrange("p (j k) -> p j k", k=2)
            so_sb = wpool.tile([F, N], f32, tag="so_sb")
            nc.vector.tensor_tensor(
                out=so_sb[:], in0=so_v[:, :, 0], in1=so_v[:, :, 1],
                op=Alu.add)
            nc.sync.dma_start(out=soutT_d[:], in_=so_sb[:])

            # ---- s_in[i, f] = sum_c hist[i,c] * T[i,c,f] ----
            t_fc = T_sb[:].rearrange("p (c f) -> p f c", c=NT)
            h_fc = hist[:].rearrange("p (o c) -> p o c", o=1) \
                          .broadcast_to([P, F, NT])
            prod = wpool.tile([P, F * NT], f32, tag="prod")
            nc.vector.tensor_tensor(
                out=prod[:], in0=t_fc, in1=h_fc, op=Alu.mult)
            sin_sb = wpool.tile([P, F], f32, tag="sin_sb")
            nc.vector.tensor_reduce(
                out=sin_sb[:], in_=prod[:].rearrange("p (f c) -> p f c", c=NT),
                axis=mybir.AxisListType.X, op=Alu.add)
            nc.sync.dma_start(out=sin_d[:], in_=sin_sb[:])

    nc.finalize()
    return nc


def _get_nc():
    if "nc" not in _CACHE:
        _CACHE["nc"] = _build_nc()
    return _CACHE["nc"]


def kernel(h, emb_table, W, b, matrix, mask):
    from concourse.bass_utils import run_bass_kernel_spmd

    h = np.asarray(h, dtype=np.float32)
    emb_table = np.asarray(emb_table, dtype=np.float32)
    W = np.asarray(W, dtype=np.float32)
    b = np.asarray(b, dtype=np.float32)
    matrix = np.asarray(matrix, dtype=np.int32)
    mask = np.asarray(mask, dtype=np.int32)

    E = emb_table @ W[H2:]                       # [NT, F]
    wstack = np.empty((H2 + 2, NT * F), np.float32)
    for c in range(NT):
        wstack[0, c * F:(c + 1) * F] = E[c]
        wstack[1:H2 + 1, c * F:(c + 1) * F] = W[:H2]
        wstack[H2 + 1, c * F:(c + 1) * F] = b

    in_maps = []
    for s in range(NCORES):
        rows = slice(s * P, (s + 1) * P)
        hx62 = np.ascontiguousarray(
            np.vstack([np.ones((1, P), np.float32), h[rows].T,
                       np.ones((1, P), np.float32)]))
        in_maps.append({
            "mat": np.ascontiguousarray(matrix[rows].reshape(P, JK)),
            "msk": np.ascontiguousarray(mask[rows].reshape(P, JK)),
            "hx62": hx62,
            "wstack": wstack,
        })

    nc = _get_nc()
    trace = bool(int(os.environ.get("KERNEL_TRACE", "0")))
    if trace:
        try:
            import ntff_shim
            ntff_shim.install()
        except Exception:
            trace = False
    res = run_bass_kernel_spmd(nc, in_maps, core_ids=list(range(NCORES)),
                               trace=trace)
    _CACHE["last_exec_ns"] = res.exec_time_ns

    s_in = np.concatenate(
        [res.results[s]["s_in_part"] for s in range(NCORES)], axis=0)
    s_out = np.sum(
        [res.results[s]["s_outT_part"] for s in range(NCORES)], axis=0).T
    return (np.ascontiguousarray(s_in),
            np.ascontiguousarray(s_out.astype(np.float32)))


# revision 12
# speedup vs baseline: 1.1896x; 1.1896x over previous
"""Trainium2 Bass kernel for nn_CalculateSLayer (GNN message passing).

Math: t[i,j,k,:] = tanh(hW[i] + E[matrix[i,j,k]] + b), E = emb @ W[60:],
masked by mask; s_in sums over (j,k), s_out over (i,k).  t depends only on
(i, c=matrix[i,j,k]) so per row i there are only 50 distinct values
T[i,c,:].  With z = mask ? matrix : 51:

  s_out[j,f] = sum_{i,c} T[i,c,f] * #{k: z[i,j,k]=c}     (PE matmuls)
  s_in[i,f]  = sum_c hist[i,c] * T[i,c,f],  hist[i,c] = #{(j,k): z=c}

Plane production is split across engines (each plane is a [128 x 2048]
bf16 image consumed by PE as a moving operand):
  * c < M:  one-hot planes (z==c) on DVE tensor_scalar, with fused
    accum_out giving hist[:,c] for free.
  * c >= M: sign planes sgn(z-c-0.5) on ACT (Sign activation) with fused
    accum_out giving cumulative count sums.  A telescoping identity turns
    sum_{c>=M} T_c*onehot_c into sum over sign planes with coefficients
    V/2 (V_{M-1}=T_M, V_c=T_{c+1}-T_c, V_49=-T_49); the coefficients sum
    to zero so the +-1 encoding needs no constant correction.
    hist[c] = (R[c-1]-R[c])/2 from the accumulated sign sums.

Rows are sharded 128 per core over 8 cores; s_out partials are summed on
the host (the unshard step of the row-sharded reduction).
"""
import os
import sys
import numpy as np

sys.path.insert(0, "/opt/trn_rl_repo")

N = 1024
H2 = 60
DEP = 10
F = 70          # DOUT
NT = 50         # edge types
NCORES = 8
P = 128         # rows per core
JK = 2 * N      # (j, k) free elements per row, k innermost
DEAD = 51.0     # masked-out elements
M = 22          # types [0, M): one-hot planes on DVE; [M, 50): steps on ACT
NSTEP = NT - M + 1   # sign planes: thresholds M-1 .. 49

_CACHE = {}


def _build_nc():
    from concourse import bacc, mybir
    from concourse import tile

    f32 = mybir.dt.float32
    bf16 = mybir.dt.bfloat16
    i32 = mybir.dt.int32
    Alu = mybir.AluOpType
    ActF = mybir.ActivationFunctionType

    nc = bacc.Bacc("TRN2", target_bir_lowering=False, debug=False,
                   num_devices=NCORES)

    mat_d = nc.dram_tensor("mat", [P, JK], i32, kind="ExternalInput")
    msk_d = nc.dram_tensor("msk", [P, JK], i32, kind="ExternalInput")
    hx62_d = nc.dram_tensor("hx62", [H2 + 2, P], f32, kind="ExternalInput")
    wstack_d = nc.dram_tensor("wstack", [H2 + 2, NT * F], f32,
                              kind="ExternalInput")

    sin_d = nc.dram_tensor("s_in_part", [P, F], f32, kind="ExternalOutput")
    soutT_d = nc.dram_tensor("s_outT_part", [F, N], f32, kind="ExternalOutput")

    with tile.TileContext(nc) as tc:
        with (
            tc.tile_pool(name="const", bufs=1) as cpool,
            tc.tile_pool(name="work", bufs=2) as wpool,
            tc.tile_pool(name="pdve", bufs=3) as pdve,
            tc.tile_pool(name="pact", bufs=3) as pact,
            tc.tile_pool(name="pbig", bufs=1, space="PSUM") as ps_big,
        ):
            # ---- inputs ----
            hx62 = cpool.tile([H2 + 2, P], f32, tag="hx62")
            wstack = cpool.tile([H2 + 2, NT * F], f32, tag="wstack")
            nc.sync.dma_start(out=hx62[:], in_=hx62_d[:])
            nc.sync.dma_start(out=wstack[:], in_=wstack_d[:])
            mat_f = wpool.tile([P, JK], f32, tag="mat_f")
            msk_f = wpool.tile([P, JK], f32, tag="msk_f")
            # SWDGE casts int32 -> f32 during the transfer
            nc.gpsimd.dma_start(out=mat_f[:], in_=mat_d[:])
            nc.gpsimd.dma_start(out=msk_f[:], in_=msk_d[:])

            # ---- T[i, c, f] = tanh(hW + b + E_c): one matmul per type,
            #      7 types per PSUM bank, tanh on ACT ----
            T_sb = cpool.tile([P, NT * F], bf16, tag="T")
            idx = 0
            while idx < NT:
                cnt = min(7, NT - idx)
                t_ps = ps_big.tile([P, 512], f32, tag="big", name=f"t_ps{idx}")
                for cl in range(cnt):
                    c = idx + cl
                    nc.tensor.matmul(
                        out=t_ps[:, cl * F:(cl + 1) * F],
                        lhsT=hx62[:], rhs=wstack[:, c * F:(c + 1) * F],
                        start=True, stop=True)
                nc.scalar.activation(
                    out=T_sb[:, idx * F:(idx + cnt) * F],
                    in_=t_ps[:, :cnt * F], func=ActF.Tanh)
                idx += cnt

            # ---- V/2 coefficients for the sign planes ----
            V2 = cpool.tile([P, NSTEP * F], bf16, tag="V2")
            dmid = cpool.tile([P, (NSTEP - 2) * F], bf16, tag="dmid")
            mid = NSTEP - 2
            nc.vector.tensor_tensor(
                out=dmid[:],
                in0=T_sb[:, (M + 1) * F:(M + 1 + mid) * F],
                in1=T_sb[:, M * F:(M + mid) * F], op=Alu.subtract)
            nc.vector.tensor_scalar(
                out=V2[:, F:(1 + mid) * F], in0=dmid[:],
                scalar1=0.5, scalar2=None, op0=Alu.mult)
            nc.vector.tensor_scalar(
                out=V2[:, 0:F], in0=T_sb[:, M * F:(M + 1) * F],
                scalar1=0.5, scalar2=None, op0=Alu.mult)
            nc.vector.tensor_scalar(
                out=V2[:, (NSTEP - 1) * F:NSTEP * F],
                in0=T_sb[:, (NT - 1) * F:NT * F],
                scalar1=-0.5, scalar2=None, op0=Alu.mult)

            # ---- z = mask ? matrix : 51  (bf16) ----
            z1 = wpool.tile([P, JK], f32, tag="z1")
            nc.vector.scalar_tensor_tensor(
                out=z1[:], in0=mat_f[:], scalar=DEAD, in1=msk_f[:],
                op0=Alu.subtract, op1=Alu.mult)
            zb = wpool.tile([P, JK], bf16, tag="zb")
            nc.vector.tensor_scalar(
                out=zb[:], in0=z1[:], scalar1=DEAD, scalar2=None,
                op0=Alu.add)

            # ---- plane loop ----
            hist = cpool.tile([P, NT], f32, tag="hist")
            rpm = cpool.tile([P, NSTEP], f32, tag="rpm")
            sbias = cpool.tile([P, NSTEP], f32, tag="sbias")
            for s in range(NSTEP):
                nc.vector.memset(sbias[:, s:s + 1], -(float(M - 1 + s) + 0.5))
            so_ps = ps_big.tile([F, N], f32, tag="big", name="so_ps")

            def consume(plane, widx, wtile, first, last):
                pv = plane[:].rearrange("p (j k) -> p j k", k=2)
                for q in range(2):
                    for k in range(2):
                        nc.tensor.matmul(
                            out=so_ps[:, q * 512:(q + 1) * 512],
                            lhsT=wtile[:, widx * F:(widx + 1) * F],
                            rhs=pv[:, q * 512:(q + 1) * 512, k],
                            start=(first and k == 0),
                            stop=(last and k == 1))

            for c in range(M):
                mc = pdve.tile([P, JK], bf16, tag="mc", name=f"mc{c}")
                nc.vector.tensor_scalar(
                    out=mc[:], in0=zb[:], scalar1=float(c), scalar2=None,
                    op0=Alu.is_equal, op1=Alu.add,
                    accum_out=hist[:, c:c + 1])
                consume(mc, c, T_sb, first=(c == 0), last=False)

            for s in range(NSTEP):
                thr = M - 1 + s  # plane = sgn(z - thr - 0.5)
                sp = pact.tile([P, JK], bf16, tag="sp", name=f"sp{s}")
                nc.scalar.activation(
                    out=sp[:], in_=zb[:], func=ActF.Sign,
                    bias=sbias[:, s:s + 1],
                    accum_out=rpm[:, s:s + 1])
                consume(sp, s, V2, first=False, last=(s == NSTEP - 1))

            # hist for c in [M, 50): (R[s-1] - R[s]) / 2 with s = c - M + 1
            hd = cpool.tile([P, NT - M], f32, tag="hd")
            nc.vector.tensor_tensor(
                out=hd[:], in0=rpm[:, 0:NT - M], in1=rpm[:, 1:NT - M + 1],
                op=Alu.subtract)
            nc.vector.tensor_scalar(
                out=hist[:, M:NT], in0=hd[:], scalar1=0.5, scalar2=None,
                op0=Alu.mult)

            # ---- s_out partial ----
            so_sb = wpool.tile([F, N], f32, tag="so_sb")
            nc.vector.tensor_copy(out=so_sb[:], in_=so_ps[:])
            nc.sync.dma_start(out=soutT_d[:], in_=so_sb[:])

            # ---- s_in[i, f] = sum_c hist[i,c] * T[i,c,f] ----
            t_fc = T_sb[:].rearrange("p (c f) -> p f c", c=NT)
            h_fc = hist[:].rearrange("p (o c) -> p o c", o=1) \
                          .broadcast_to([P, F, NT])
            prod = wpool.tile([P, F * NT], f32, tag="prod")
            nc.vector.tensor_tensor(
                out=prod[:], in0=t_fc, in1=h_fc, op=Alu.mult)
            sin_sb = wpool.tile([P, F], f32, tag="sin_sb")
            nc.vector.tensor_reduce(
                out=sin_sb[:], in_=prod[:].rearrange("p (f c) -> p f c", c=NT),
                axis=mybir.AxisListType.X, op=Alu.add)
            nc.sync.dma_start(out=sin_d[:], in_=sin_sb[:])

    nc.finalize()
    return nc


def _get_nc():
    if "nc" not in _CACHE:
        _CACHE["nc"] = _build_nc()
    return _CACHE["nc"]


def kernel(h, emb_table, W, b, matrix, mask):
    from concourse.bass_utils import run_bass_kernel_spmd

    h = np.asarray(h, dtype=np.float32)
    emb_table = np.asarray(emb_table, dtype=np.float32)
    W = np.asarray(W, dtype=np.float32)
    b = np.asarray(b, dtype=np.float32)
    matrix = np.asarray(matrix, dtype=np.int32)
    mask = np.asarray(mask, dtype=np.int32)

    E = emb_table @ W[H2:]                       # [NT, F]
    wstack = np.empty((H2 + 2, NT * F), np.float32)
    for c in range(NT):
        wstack[0, c * F:(c + 1) * F] = E[c]
        wstack[1:H2 + 1, c * F:(c + 1) * F] = W[:H2]
        wstack[H2 + 1, c * F:(c + 1) * F] = b

    in_maps = []
    for s in range(NCORES):
        rows = slice(s * P, (s + 1) * P)
        hx62 = np.ascontiguousarray(
            np.vstack([np.ones((1, P), np.float32), h[rows].T,
                       np.ones((1, P), np.float32)]))
        in_maps.append({
            "mat": np.ascontiguousarray(matrix[rows].reshape(P, JK)),
            "msk": np.ascontiguousarray(mask[rows].reshape(P, JK)),
            "hx62": hx62,
            "wstack": wstack,
        })

    nc = _get_nc()
    trace = bool(int(os.environ.get("KERNEL_TRACE", "0")))
    if trace:
        try:
            import ntff_shim
            ntff_shim.install()
        except Exception:
            trace = False
    res = run_bass_kernel_spmd(nc, in_maps, core_ids=list(range(NCORES)),
                               trace=trace)
    _CACHE["last_exec_ns"] = res.exec_time_ns

    s_in = np.concatenate(
        [res.results[s]["s_in_part"] for s in range(NCORES)], axis=0)
    s_out = np.sum(
        [res.results[s]["s_outT_part"] for s in range(NCORES)], axis=0).T
    return (np.ascontiguousarray(s_in),
            np.ascontiguousarray(s_out.astype(np.float32)))
